# revision 28
# baseline (speedup 1.0000x reference)
"""Multi-head causal attention (B=4, T=2048, DM=1024, H=16, DK=DV=64) on 8 TRN2
NeuronCores.

Sharding: hybrid batch x head-group. Core c owns batch c//2 and head-group
c%2 (8 heads). Each core:
  1. keeps x^T for its batch resident in SBUF (fp16), projects Q^T/K^T per
     head-pair plus V in natural [token, dim] layout (no transpose needed),
  2. runs causal attention per head-pair in S^T = K Q^T layout with
     block-skipping; the causal mask is pre-written into PSUM by a PE
     matmul so the QK^T accumulates on top of it (keeps DVE off the
     S->exp->PV critical path); softmax denominators ride along as a
     ones-column appended to V; normalization uses a K=1 matmul broadcast
     + reciprocal_approx_fast,
  3. contracts its 8 heads against its 512 rows of W_o and writes a
     [T, DM] partial straight from PSUM.
Host sums the 2 partials per batch and adds the bias.
"""

import sys

for _p in ("/opt/trn_rl_repo",):
    if _p not in sys.path:
        sys.path.insert(0, _p)

import numpy as np

# ---- problem constants (hardcoded per harness contract) ----
B, T, DM = 4, 2048, 1024
H, DK = 16, 64
NCORES = 8
HPG = 8                     # heads per core (head-group)
NHP = 4                     # head-pairs per core
SD = HPG * DK               # 512: this core's slice of the concat-head dim
ND = DM // 128              # contraction k-tiles for projections
NT = T // 128               # s-tiles per batch
PW = 1024                   # attention q "pair" width (PSUM S tile free size)
CH = 512                    # PSUM chunk / matmul moving width
NPAIR = T // PW
SCALE = DK ** -0.5
MASK = -60000.0             # additive causal mask (fp16-representable)

_CACHE = {}


def _build():
    import concourse.bass as bass
    import concourse.tile as tile
    from concourse import bacc, mybir

    f32 = mybir.dt.float32
    f32r = mybir.dt.float32r
    f16 = mybir.dt.float16
    ts = bass.ts

    nc = bacc.Bacc("TRN2", target_bir_lowering=False, debug=False,
                   num_devices=NCORES)

    xb = nc.dram_tensor("xb", [DM, T], f16, kind="ExternalInput").ap()
    wq2 = nc.dram_tensor("wq2", [DM, SD], f16, kind="ExternalInput").ap()
    wk2 = nc.dram_tensor("wk2", [DM, SD], f16, kind="ExternalInput").ap()
    wv2 = nc.dram_tensor("wv2", [DM, SD], f16, kind="ExternalInput").ap()
    wo8 = nc.dram_tensor("wo8", [SD, DM], f16, kind="ExternalInput").ap()
    outp = nc.dram_tensor("out_part", [T, DM], f32, kind="ExternalOutput").ap()

    # additive causal mask in S^T layout: MASK where q < s (strict lower)
    tri = ((1.0 - np.triu(np.ones((128, 128)))) * MASK).astype(np.float16)
    ident = np.eye(128, dtype=np.float16)
    # broadcast row lives at partition 64 so its base partition matches the
    # reciprocal-denominator row it broadcasts (matmul requires equal bases)
    ones2 = np.zeros((65, 128), np.float32)
    ones2[64, :] = 1.0
    tri_h = nc.inline_tensor(tri, name="tri_const")
    id_h = nc.inline_tensor(ident, name="id_const")
    ones2_h = nc.inline_tensor(ones2, name="ones2_const")

    with tile.TileContext(nc) as tc:
        with (
            tc.tile_pool(name="singles", bufs=1) as singles,
            tc.tile_pool(name="seq", bufs=2) as seq,
            tc.tile_pool(name="att", bufs=3) as att,
            tc.tile_pool(name="small", bufs=2) as small,
            tc.tile_pool(name="ps", bufs=2, space="PSUM") as ps,
            tc.tile_pool(name="pacc", bufs=2, space="PSUM") as pacc,
            tc.tile_pool(name="paux", bufs=2, space="PSUM") as paux,
        ):
            # ---- constants into SBUF ----
            tri_sb = singles.tile([128, 128], f16, tag="tri")
            nc.sync.dma_start(out=tri_sb, in_=tri_h.ap())
            id_sb = singles.tile([128, 128], f16, tag="id")
            nc.sync.dma_start(out=id_sb, in_=id_h.ap())
            ones2_sb = singles.tile([65, 128], f32r, tag="ones2")
            nc.gpsimd.dma_start(out=ones2_sb, in_=ones2_h.ap())
            # ---- resident inputs (wq + first x block first: shortest
            # path to the first projection matmul; per-(a, tblock) x chunks
            # are 1KB-contiguous per partition for efficient descriptors) ----
            x_sb = singles.tile([128, ND, T], f16, tag="x")
            xr = xb.rearrange("(a p) t -> p a t", p=128)
            w_sb = {}
            for nm, src in (("q", wq2), ("k", wk2), ("v", wv2)):
                w_sb[nm] = singles.tile([128, ND, SD], f16, tag=f"w{nm}",
                                        name=f"w{nm}_sb")
            wrr = {"q": wq2, "k": wk2, "v": wv2}
            wrr = {nm: wrr[nm].rearrange("(a p) m -> p a m", p=128)
                   for nm in wrr}
            for a in range(ND):
                nc.sync.dma_start(out=w_sb["q"][:, a, :], in_=wrr["q"][:, a, :])
                nc.sync.dma_start(out=x_sb[:, a, ts(0, 512)],
                                  in_=xr[:, a, ts(0, 512)])
            for a in range(ND):
                nc.sync.dma_start(out=x_sb[:, a, ts(1, 512)],
                                  in_=xr[:, a, ts(1, 512)])
                nc.sync.dma_start(out=w_sb["k"][:, a, :], in_=wrr["k"][:, a, :])
            for a in range(ND):
                nc.sync.dma_start(out=x_sb[:, a, ts(2, 512)],
                                  in_=xr[:, a, ts(2, 512)])
                nc.sync.dma_start(out=w_sb["v"][:, a, :], in_=wrr["v"][:, a, :])
            for a in range(ND):
                nc.sync.dma_start(out=x_sb[:, a, ts(3, 512)],
                                  in_=xr[:, a, ts(3, 512)])
            wo_sb = singles.tile([128, NHP, DM], f16, tag="wo")
            nc.sync.dma_start(
                out=wo_sb, in_=wo8.rearrange("(a p) m -> p a m", p=128))

            # concat-head attention output, [hv, hp, tok]
            onorm = singles.tile([128, NHP, T], f16, tag="onorm")

            # Projections are emitted in 4 pieces interleaved into the
            # PREVIOUS head-pair's attention so the scheduler always has
            # dense PE filler work available while attention waits on exp.
            # kt holds both heads [128, T]; it doubles as the K=128
            # stationary for S (the zero rows of the padded q moving
            # operand select the head). qt is split per head with the
            # other head's partitions zeroed so the moving operand spans
            # all 128 partitions (full SBUF port bandwidth + FWL).
            proj_t = {}

            def emit_proj(hp, piece):
                # pieces 0/1/2: q/k/v for tokens 0:1024 (enough to start
                # pair p=0 of this head-pair); pieces 3/4/5: the rest.
                c_lo, c_hi = hp * 128, (hp + 1) * 128
                if piece == 0:
                    proj_t[hp] = (
                        seq.tile([128, T], f16, tag="kt", name="kt"),
                        [seq.tile([128, T], f16, tag=f"qt{h}", name=f"qt{h}")
                         for h in (0, 1)],
                        seq.tile([128, NT * 256], f16, tag="vsb",
                                 name="vsb"),
                    )
                kt, qth, vsb = proj_t[hp]
                if piece in (0, 3):
                    if piece == 0:
                        nc.gpsimd.memset(qth[0][64:128, :], 0.0)
                        nc.gpsimd.memset(qth[1][0:64, :], 0.0)
                    for tb in ((0, 1) if piece == 0 else (2, 3)):
                        pj = paux.tile([128, CH], f32, tag="aux")
                        for a in range(ND):
                            nc.tensor.matmul(
                                pj, w_sb["q"][:, a, c_lo:c_hi],
                                x_sb[:, a, ts(tb, 512)],
                                start=(a == 0), stop=(a == ND - 1))
                        nc.vector.tensor_copy(
                            qth[0][0:64, ts(tb, 512)], pj[0:64, :])
                        nc.vector.tensor_copy(
                            qth[1][64:128, ts(tb, 512)], pj[64:128, :])
                elif piece in (1, 4):
                    for tb in ((0, 1) if piece == 1 else (2, 3)):
                        pj = paux.tile([128, CH], f32, tag="aux")
                        for a in range(ND):
                            nc.tensor.matmul(
                                pj, w_sb["k"][:, a, c_lo:c_hi],
                                x_sb[:, a, ts(tb, 512)],
                                start=(a == 0), stop=(a == ND - 1))
                        nc.vector.tensor_copy(kt[:, ts(tb, 512)], pj)
                else:
                    # V in [token, dim] layout; per s-tile j and head h a
                    # 128-col block [v(64) | 1 | zeros(63)] so the PV
                    # stationary is a full 128x128 operand.
                    vsb4 = vsb.rearrange("p (n h c) -> p n h c", h=2, c=128)
                    if piece == 2:
                        nc.gpsimd.memset(vsb4[:, :, :, 64:128], 0.0)
                        nc.gpsimd.memset(vsb4[:, :, :, 64:65], 1.0)
                    for tcj in range(8) if piece == 2 else range(8, NT):
                        pv = paux.tile([128, CH], f32, tag="aux")
                        for a in range(ND):
                            nc.tensor.matmul(
                                pv[:, 0:128], x_sb[:, a, ts(tcj, 128)],
                                w_sb["v"][:, a, c_lo:c_hi],
                                start=(a == 0), stop=(a == ND - 1))
                        nc.vector.tensor_copy(
                            vsb4[:, tcj, :, 0:64],
                            pv[:, 0:128].rearrange("p (h c) -> p h c", h=2))

            def emit_wo(tcis):
                for tci in tcis:
                    for cc in range(DM // CH):
                        po = paux.tile([128, CH], f32, tag="aux")
                        for hp2 in range(NHP):
                            nc.tensor.matmul(
                                po, onorm[:, hp2, ts(tci, 128)],
                                wo_sb[:, hp2, ts(cc, CH)],
                                start=(hp2 == 0), stop=(hp2 == NHP - 1))
                        osb = att.tile([128, CH], f32, tag="osb", name="osb")
                        nc.vector.tensor_copy(osb, po)
                        nc.sync.dma_start(
                            out=outp[tci * 128:(tci + 1) * 128, ts(cc, CH)],
                            in_=osb)

            for piece in range(3):
                emit_proj(0, piece)

            for hp in range(NHP):
                kt, qth, vsb = proj_t[hp]
                # h1's normalized rows are produced at base partition 0 (DVE
                # lanes are partition-locked) and DMA'd to partitions 64..127
                # of onorm at the end of each pair.
                onorm1 = seq.tile([64, T], f16, tag="onorm1")
                for p in range(NPAIR):
                    nj = (p + 1) * (PW // 128)
                    for h in (0, 1):
                        acc = [pacc.tile([128, CH], f32, tag="acc",
                                         name="acc")
                               for _ in range(PW // CH)]

                        def normalize(qq, h=h, p=p, acc=acc, onorm1=onorm1):
                            # O' rows 0..63, denominator row 64; emitted
                            # right after the last contributing PV so the
                            # acc slot frees as early as possible.
                            dsb = small.tile([65, CH], f32r, tag="dsb")
                            nc.vector.tensor_copy(
                                dsb[64:65, :], acc[qq][64:65, :])
                            dbc = paux.tile([128, CH], f32, tag="aux")
                            nc.tensor.matmul(
                                dbc, ones2_sb[64:65, :], dsb[64:65, :],
                                start=True, stop=True)
                            rcp = small.tile([64, CH], f32, tag="rcp")
                            nc.vector.reciprocal_approx_fast(
                                out=rcp, in_=dbc[0:64, :])
                            q_lo = p * PW + qq * CH
                            if h == 0:
                                nc.vector.tensor_mul(
                                    onorm[0:64, hp, q_lo:q_lo + CH],
                                    acc[qq][0:64, :], rcp)
                            else:
                                nc.vector.tensor_mul(
                                    onorm1[:, q_lo:q_lo + CH],
                                    acc[qq][0:64, :], rcp)

                        for j in range(nj):
                            j_rel = j - p * (PW // 128)
                            c0 = max(0, 128 * j_rel)
                            S = ps.tile([128, PW], f32, tag="s")
                            if j_rel >= 0:  # diagonal: mask pre-written by PE
                                nc.tensor.matmul(
                                    S[:, c0:c0 + 128], id_sb, tri_sb,
                                    start=True, stop=False,
                                    skip_group_check=True)
                                nc.tensor.matmul(
                                    S[:, c0:c0 + 128],
                                    kt[:, ts(j, 128)],
                                    qth[h][:,
                                           p * PW + c0: p * PW + c0 + 128],
                                    start=False, stop=True,
                                    skip_group_check=True)
                                col = c0 + 128
                            else:
                                col = 0
                            while col < PW:
                                hi = min(PW, (col // CH + 1) * CH)
                                nc.tensor.matmul(
                                    S[:, col:hi],
                                    kt[:, ts(j, 128)],
                                    qth[h][:, p * PW + col: p * PW + hi],
                                    start=True, stop=True,
                                    skip_group_check=True)
                                col = hi
                            E = att.tile([128, PW], f16, tag="expt")
                            nc.scalar.activation(
                                out=E[:, c0:PW], in_=S[:, c0:PW],
                                func=mybir.ActivationFunctionType.Exp,
                                scale=SCALE)
                            for qq in range(PW // CH):
                                lo = max(c0, qq * CH)
                                if lo >= (qq + 1) * CH:
                                    continue
                                last = nj - 1 if qq > 0 else \
                                    min(nj - 1, p * (PW // 128) + 3)
                                nc.tensor.matmul(
                                    acc[qq][:, lo - qq * CH: CH],
                                    vsb[:, j * 256 + h * 128:
                                        j * 256 + h * 128 + 128],
                                    E[:, lo:(qq + 1) * CH],
                                    start=(j == 0), stop=(j == last),
                                    skip_group_check=True)
                                if j == last:
                                    normalize(qq)
                        # fill attention's exp-wait gaps with projection
                        # work: this hp's second token-half during p0, the
                        # next hp's first token-half during p1.
                        b = 2 * p + h
                        if b == 0:
                            emit_proj(hp, 3)
                            emit_proj(hp, 4)
                        elif b == 1:
                            emit_proj(hp, 5)
                        elif hp + 1 < NHP:
                            if b == 2:
                                emit_proj(hp + 1, 0)
                                emit_proj(hp + 1, 1)
                            else:
                                emit_proj(hp + 1, 2)
                    # place this pair's h1 rows at partitions 64..127 (DMA
                    # moves across partitions; DVE cannot)
                    nc.sync.dma_start(
                        out=onorm[64:128, hp, p * PW:(p + 1) * PW],
                        in_=onorm1[:, p * PW:(p + 1) * PW])
                    if hp == NHP - 1 and p == 0:
                        emit_wo(range(0, 8))

            # ======= rest of row-sharded W_o =======
            emit_wo(range(8, NT))

    nc.compile()
    return nc


def _get_nc():
    if "nc" not in _CACHE:
        _CACHE["nc"] = _build()
    return _CACHE["nc"]


def make_in_maps(x, Wq, Wk, Wv, Wo, bo):
    x2d = np.asarray(x, dtype=np.float32).reshape(B * T, DM)
    xT = np.ascontiguousarray(x2d.T.astype(np.float16))       # [DM, BT]
    xbs = [np.ascontiguousarray(xT[:, b * T:(b + 1) * T]) for b in range(B)]
    wmaps = []
    for g in range(2):
        hs = range(g * HPG, (g + 1) * HPG)
        wmaps.append({
            "wq2": np.ascontiguousarray(np.concatenate(
                [Wq[h] for h in hs], 1).astype(np.float16)),
            "wk2": np.ascontiguousarray(np.concatenate(
                [Wk[h] for h in hs], 1).astype(np.float16)),
            "wv2": np.ascontiguousarray(np.concatenate(
                [Wv[h] for h in hs], 1).astype(np.float16)),
            "wo8": np.ascontiguousarray(
                np.asarray(Wo[g * SD:(g + 1) * SD]).astype(np.float16)),
        })
    maps = []
    for c in range(NCORES):
        b, g = c // 2, c % 2
        m = {"xb": xbs[b]}
        m.update(wmaps[g])
        maps.append(m)
    return maps


def run(x, Wq, Wk, Wv, Wo, bo, trace=False, **spmd_kwargs):
    from concourse.bass_utils import run_bass_kernel_spmd

    nc = _get_nc()
    maps = make_in_maps(x, Wq, Wk, Wv, Wo, bo)
    res = run_bass_kernel_spmd(
        nc, maps, core_ids=list(range(NCORES)), trace=trace, **spmd_kwargs
    )
    out = np.empty((B, T, DM), np.float32)
    bof = np.asarray(bo, dtype=np.float32)
    for b in range(B):
        out[b] = res.results[2 * b]["out_part"] \
            + res.results[2 * b + 1]["out_part"] + bof
    return out, res


def kernel(x, Wq, Wk, Wv, Wo, bo):
    out, _ = run(x, Wq, Wk, Wv, Wo, bo)
    return out


# revision 29
# speedup vs baseline: 1.0275x; 1.0275x over previous
"""Multi-head causal attention (B=4, T=2048, DM=1024, H=16, DK=DV=64) on 8 TRN2
NeuronCores.

Sharding: hybrid batch x head-group. Core c owns batch c//2 and head-group
c%2 (8 heads). Each core:
  1. keeps x^T for its batch resident in SBUF (fp16), projects Q^T/K^T per
     head-pair plus V in natural [token, dim] layout (no transpose needed),
  2. runs causal attention per head-pair in S^T = K Q^T layout with
     block-skipping; the causal mask is pre-written into PSUM by a PE
     matmul so the QK^T accumulates on top of it (keeps DVE off the
     S->exp->PV critical path); softmax denominators ride along as a
     ones-column appended to V; normalization uses a K=1 matmul broadcast
     + reciprocal_approx_fast,
  3. contracts its 8 heads against its 512 rows of W_o and writes a
     [T, DM] partial straight from PSUM.
Host sums the 2 partials per batch and adds the bias.
"""

import sys

for _p in ("/opt/trn_rl_repo",):
    if _p not in sys.path:
        sys.path.insert(0, _p)

import numpy as np

# ---- problem constants (hardcoded per harness contract) ----
B, T, DM = 4, 2048, 1024
H, DK = 16, 64
NCORES = 8
HPG = 8                     # heads per core (head-group)
NHP = 4                     # head-pairs per core
SD = HPG * DK               # 512: this core's slice of the concat-head dim
ND = DM // 128              # contraction k-tiles for projections
NT = T // 128               # s-tiles per batch
PW = 1024                   # attention q "pair" width (PSUM S tile free size)
CH = 512                    # PSUM chunk / matmul moving width
NPAIR = T // PW
SCALE = DK ** -0.5
MASK = -60000.0             # additive causal mask (fp16-representable)

_CACHE = {}


def _build():
    import concourse.bass as bass
    import concourse.tile as tile
    from concourse import bacc, mybir

    f32 = mybir.dt.float32
    f32r = mybir.dt.float32r
    f16 = mybir.dt.float16
    ts = bass.ts

    nc = bacc.Bacc("TRN2", target_bir_lowering=False, debug=False,
                   num_devices=NCORES)

    xb = nc.dram_tensor("xb", [DM, T], f16, kind="ExternalInput").ap()
    wq2 = nc.dram_tensor("wq2", [DM, SD], f16, kind="ExternalInput").ap()
    wk2 = nc.dram_tensor("wk2", [DM, SD], f16, kind="ExternalInput").ap()
    wv2 = nc.dram_tensor("wv2", [DM, SD], f16, kind="ExternalInput").ap()
    wo8 = nc.dram_tensor("wo8", [SD, DM], f16, kind="ExternalInput").ap()
    outp = nc.dram_tensor("out_part", [T, DM], f32, kind="ExternalOutput").ap()

    # additive causal mask in S^T layout: MASK where q < s (strict lower)
    tri = ((1.0 - np.triu(np.ones((128, 128)))) * MASK).astype(np.float16)
    ident = np.eye(128, dtype=np.float16)
    # broadcast row lives at partition 64 so its base partition matches the
    # accumulator denominator row it multiplies (matmul requires equal bases)
    ones2 = np.zeros((65, 128), np.float32)
    ones2[64, :] = 1.0
    tri_h = nc.inline_tensor(tri, name="tri_const")
    id_h = nc.inline_tensor(ident, name="id_const")
    ones2_h = nc.inline_tensor(ones2, name="ones2_const")

    with tile.TileContext(nc) as tc:
        with (
            tc.tile_pool(name="singles", bufs=1) as singles,
            tc.tile_pool(name="seq", bufs=2) as seq,
            tc.tile_pool(name="att", bufs=4) as att,
            tc.tile_pool(name="small", bufs=3) as small,
            tc.tile_pool(name="ps", bufs=2, space="PSUM") as ps,
            tc.tile_pool(name="pacc", bufs=2, space="PSUM") as pacc,
            tc.tile_pool(name="paux", bufs=2, space="PSUM") as paux,
        ):
            # ---- constants into SBUF ----
            tri_sb = singles.tile([128, 128], f16, tag="tri")
            nc.sync.dma_start(out=tri_sb, in_=tri_h.ap())
            id_sb = singles.tile([128, 128], f16, tag="id")
            nc.sync.dma_start(out=id_sb, in_=id_h.ap())
            ones2_sb = singles.tile([65, 128], f32r, tag="ones2")
            nc.gpsimd.dma_start(out=ones2_sb, in_=ones2_h.ap())

            # ---- resident inputs (wq + first x block first: shortest
            # path to the first projection matmul; per-(a, tblock) x chunks
            # are 1KB-contiguous per partition for efficient descriptors) ----
            x_sb = singles.tile([128, ND, T], f16, tag="x")
            xr = xb.rearrange("(a p) t -> p a t", p=128)
            w_sb = {}
            for nm, src in (("q", wq2), ("k", wk2), ("v", wv2)):
                w_sb[nm] = singles.tile([128, ND, SD], f16, tag=f"w{nm}",
                                        name=f"w{nm}_sb")
            wr = {"q": wq2, "k": wk2, "v": wv2}
            nc.sync.dma_start(
                out=w_sb["q"], in_=wr["q"].rearrange("(a p) m -> p a m", p=128))
            for tb in range(4):
                for a in range(ND):
                    nc.sync.dma_start(out=x_sb[:, a, ts(tb, 512)],
                                      in_=xr[:, a, ts(tb, 512)])
                if tb == 0:
                    nc.sync.dma_start(
                        out=w_sb["k"],
                        in_=wr["k"].rearrange("(a p) m -> p a m", p=128))
                if tb == 1:
                    nc.sync.dma_start(
                        out=w_sb["v"],
                        in_=wr["v"].rearrange("(a p) m -> p a m", p=128))
            wo_sb = singles.tile([128, NHP, DM], f16, tag="wo")
            nc.sync.dma_start(
                out=wo_sb, in_=wo8.rearrange("(a p) m -> p a m", p=128))

            # concat-head attention output, [hv, hp, tok]
            onorm = singles.tile([128, NHP, T], f16, tag="onorm")

            # Projections are emitted in 4 pieces interleaved into the
            # PREVIOUS head-pair's attention so the scheduler always has
            # dense PE filler work available while attention waits on exp.
            # kt holds both heads [128, T]; it doubles as the K=128
            # stationary for S (the zero rows of the padded q moving
            # operand select the head). qt is split per head with the
            # other head's partitions zeroed so the moving operand spans
            # all 128 partitions (full SBUF port bandwidth + FWL).
            proj_t = {}

            def emit_proj(hp, piece):
                # pieces 0/1/2: q/k/v for tokens 0:1024 (enough to start
                # pair p=0 of this head-pair); pieces 3/4/5: the rest.
                c_lo, c_hi = hp * 128, (hp + 1) * 128
                if piece == 0:
                    proj_t[hp] = (
                        seq.tile([128, T], f16, tag="kt", name="kt"),
                        [seq.tile([128, T], f16, tag=f"qt{h}", name=f"qt{h}")
                         for h in (0, 1)],
                        seq.tile([128, NT * 256], f16, tag="vsb",
                                 name="vsb"),
                    )
                kt, qth, vsb = proj_t[hp]
                if piece in (0, 3):
                    if piece == 0:
                        nc.gpsimd.memset(qth[0][64:128, :], 0.0)
                        nc.gpsimd.memset(qth[1][0:64, :], 0.0)
                    for tb in ((0, 1) if piece == 0 else (2, 3)):
                        pj = paux.tile([128, CH], f32, tag="aux")
                        for a in range(ND):
                            nc.tensor.matmul(
                                pj, w_sb["q"][:, a, c_lo:c_hi],
                                x_sb[:, a, ts(tb, 512)],
                                start=(a == 0), stop=(a == ND - 1))
                        nc.vector.tensor_copy(
                            qth[0][0:64, ts(tb, 512)], pj[0:64, :])
                        nc.vector.tensor_copy(
                            qth[1][64:128, ts(tb, 512)], pj[64:128, :])
                elif piece in (1, 4):
                    for tb in ((0, 1) if piece == 1 else (2, 3)):
                        pj = paux.tile([128, CH], f32, tag="aux")
                        for a in range(ND):
                            nc.tensor.matmul(
                                pj, w_sb["k"][:, a, c_lo:c_hi],
                                x_sb[:, a, ts(tb, 512)],
                                start=(a == 0), stop=(a == ND - 1))
                        nc.vector.tensor_copy(kt[:, ts(tb, 512)], pj)
                else:
                    # V in [token, dim] layout; per s-tile j and head h a
                    # 128-col block [v(64) | 1 | zeros(63)] so the PV
                    # stationary is a full 128x128 operand.
                    vsb4 = vsb.rearrange("p (n h c) -> p n h c", h=2, c=128)
                    if piece == 2:
                        nc.gpsimd.memset(vsb4[:, :, :, 64:128], 0.0)
                        nc.gpsimd.memset(vsb4[:, :, :, 64:65], 1.0)
                    for tcj in range(8) if piece == 2 else range(8, NT):
                        pv = paux.tile([128, CH], f32, tag="aux")
                        for a in range(ND):
                            nc.tensor.matmul(
                                pv[:, 0:128], x_sb[:, a, ts(tcj, 128)],
                                w_sb["v"][:, a, c_lo:c_hi],
                                start=(a == 0), stop=(a == ND - 1))
                        nc.vector.tensor_copy(
                            vsb4[:, tcj, :, 0:64],
                            pv[:, 0:128].rearrange("p (h c) -> p h c", h=2))

            def emit_wo(tcis):
                for tci in tcis:
                    for cc in range(DM // CH):
                        po = paux.tile([128, CH], f32, tag="aux")
                        for hp2 in range(NHP):
                            nc.tensor.matmul(
                                po, onorm[:, hp2, ts(tci, 128)],
                                wo_sb[:, hp2, ts(cc, CH)],
                                start=(hp2 == 0), stop=(hp2 == NHP - 1))
                        osb = att.tile([128, CH], f32, tag="osb", name="osb")
                        nc.vector.tensor_copy(osb, po)
                        nc.sync.dma_start(
                            out=outp[tci * 128:(tci + 1) * 128, ts(cc, CH)],
                            in_=osb)

            for piece in range(3):
                emit_proj(0, piece)

            for hp in range(NHP):
                kt, qth, vsb = proj_t[hp]
                # h1's normalized rows are produced at base partition 0 (DVE
                # lanes are partition-locked) and DMA'd to partitions 64..127
                # of onorm at the end of each pair.
                onorm1 = seq.tile([64, T], f16, tag="onorm1")
                for p in range(NPAIR):
                    nj = (p + 1) * (PW // 128)
                    for h in (0, 1):
                        acc = [pacc.tile([128, CH], f32, tag="acc",
                                         name="acc")
                               for _ in range(PW // CH)]

                        def normalize(qq, h=h, p=p, acc=acc, onorm1=onorm1):
                            # O' rows 0..63, denominator row 64; emitted
                            # right after the last contributing PV so the
                            # acc slot frees as early as possible.
                            dsb = small.tile([65, CH], f32r, tag="dsb")
                            nc.vector.tensor_copy(
                                dsb[64:65, :], acc[qq][64:65, :])
                            dbc = paux.tile([128, CH], f32, tag="aux")
                            nc.tensor.matmul(
                                dbc, ones2_sb[64:65, :], dsb[64:65, :],
                                start=True, stop=True)
                            rcp = small.tile([64, CH], f32, tag="rcp")
                            nc.vector.reciprocal_approx_fast(
                                out=rcp, in_=dbc[0:64, :])
                            q_lo = p * PW + qq * CH
                            if h == 0:
                                nc.vector.tensor_mul(
                                    onorm[0:64, hp, q_lo:q_lo + CH],
                                    acc[qq][0:64, :], rcp)
                            else:
                                nc.vector.tensor_mul(
                                    onorm1[:, q_lo:q_lo + CH],
                                    acc[qq][0:64, :], rcp)

                        for j in range(nj):
                            j_rel = j - p * (PW // 128)
                            c0 = max(0, 128 * j_rel)
                            S = ps.tile([128, PW], f32, tag="s")
                            if j_rel >= 0:  # diagonal: mask pre-written by PE
                                nc.tensor.matmul(
                                    S[:, c0:c0 + 128], id_sb, tri_sb,
                                    start=True, stop=False,
                                    skip_group_check=True)
                                nc.tensor.matmul(
                                    S[:, c0:c0 + 128],
                                    kt[:, ts(j, 128)],
                                    qth[h][:,
                                           p * PW + c0: p * PW + c0 + 128],
                                    start=False, stop=True,
                                    skip_group_check=True)
                                col = c0 + 128
                            else:
                                col = 0
                            while col < PW:
                                hi = min(PW, (col // CH + 1) * CH)
                                nc.tensor.matmul(
                                    S[:, col:hi],
                                    kt[:, ts(j, 128)],
                                    qth[h][:, p * PW + col: p * PW + hi],
                                    start=True, stop=True,
                                    skip_group_check=True)
                                col = hi
                            E = att.tile([128, PW], f16, tag="expt")
                            nc.scalar.activation(
                                out=E[:, c0:PW], in_=S[:, c0:PW],
                                func=mybir.ActivationFunctionType.Exp,
                                scale=SCALE)
                            for qq in range(PW // CH):
                                lo = max(c0, qq * CH)
                                if lo >= (qq + 1) * CH:
                                    continue
                                last = nj - 1 if qq > 0 else \
                                    min(nj - 1, p * (PW // 128) + 3)
                                nc.tensor.matmul(
                                    acc[qq][:, lo - qq * CH: CH],
                                    vsb[:, j * 256 + h * 128:
                                        j * 256 + h * 128 + 128],
                                    E[:, lo:(qq + 1) * CH],
                                    start=(j == 0), stop=(j == last),
                                    skip_group_check=True)
                                if j == last:
                                    normalize(qq)
                        # fill attention's exp-wait gaps with projection
                        # work: this hp's second token-half during p0, the
                        # next hp's first token-half during p1.
                        b = 2 * p + h
                        if b == 0:
                            emit_proj(hp, 3)
                            emit_proj(hp, 4)
                        elif b == 1:
                            emit_proj(hp, 5)
                        elif hp + 1 < NHP:
                            if b == 2:
                                emit_proj(hp + 1, 0)
                                emit_proj(hp + 1, 1)
                            else:
                                emit_proj(hp + 1, 2)
                    # place this pair's h1 rows at partitions 64..127 (DMA
                    # moves across partitions; DVE cannot)
                    nc.sync.dma_start(
                        out=onorm[64:128, hp, p * PW:(p + 1) * PW],
                        in_=onorm1[:, p * PW:(p + 1) * PW])
                    if hp == NHP - 1 and p == 0:
                        emit_wo(range(0, 8))

            # ======= rest of row-sharded W_o =======
            emit_wo(range(8, NT))

    nc.compile()
    return nc


def _get_nc():
    if "nc" not in _CACHE:
        _CACHE["nc"] = _build()
    return _CACHE["nc"]


def make_in_maps(x, Wq, Wk, Wv, Wo, bo):
    x2d = np.asarray(x, dtype=np.float32).reshape(B * T, DM)
    xT = np.ascontiguousarray(x2d.T.astype(np.float16))       # [DM, BT]
    xbs = [np.ascontiguousarray(xT[:, b * T:(b + 1) * T]) for b in range(B)]
    wmaps = []
    for g in range(2):
        hs = range(g * HPG, (g + 1) * HPG)
        wmaps.append({
            "wq2": np.ascontiguousarray(np.concatenate(
                [Wq[h] for h in hs], 1).astype(np.float16)),
            "wk2": np.ascontiguousarray(np.concatenate(
                [Wk[h] for h in hs], 1).astype(np.float16)),
            "wv2": np.ascontiguousarray(np.concatenate(
                [Wv[h] for h in hs], 1).astype(np.float16)),
            "wo8": np.ascontiguousarray(
                np.asarray(Wo[g * SD:(g + 1) * SD]).astype(np.float16)),
        })
    maps = []
    for c in range(NCORES):
        b, g = c // 2, c % 2
        m = {"xb": xbs[b]}
        m.update(wmaps[g])
        maps.append(m)
    return maps


def run(x, Wq, Wk, Wv, Wo, bo, trace=False, **spmd_kwargs):
    from concourse.bass_utils import run_bass_kernel_spmd

    nc = _get_nc()
    maps = make_in_maps(x, Wq, Wk, Wv, Wo, bo)
    res = run_bass_kernel_spmd(
        nc, maps, core_ids=list(range(NCORES)), trace=trace, **spmd_kwargs
    )
    out = np.empty((B, T, DM), np.float32)
    bof = np.asarray(bo, dtype=np.float32)
    for b in range(B):
        out[b] = res.results[2 * b]["out_part"] \
            + res.results[2 * b + 1]["out_part"] + bof
    return out, res


def kernel(x, Wq, Wk, Wv, Wo, bo):
    out, _ = run(x, Wq, Wk, Wv, Wo, bo)
    return out
